# revision 1
# baseline (speedup 1.0000x reference)
"""DecoderRNN (attention LSTM decoder, greedy decode) on 8 Trainium2 NeuronCores.

Strategy: the T=128 step recurrence is sequential; run it SPMD on all 8 cores
with tensor-parallel sharding — vocab dim of out_w/log-softmax split 8 ways,
LSTM gate/hidden dims split 8 ways, attention batch-sharded (2 rows/core) —
and 4 small AllGathers per step (ctx, h0, h1, argmax/logZ info). All matmuls
fp32 on the PE; log_softmax written in-loop from each core's vocab shard.
"""
import numpy as np

import concourse.bass as bass
import concourse.bacc as bacc
import concourse.mybir as mybir
import concourse.tile as tile
from concourse.bass_utils import run_bass_kernel_spmd

AF = mybir.ActivationFunctionType
ALU = mybir.AluOpType
F32 = mybir.dt.float32
U32 = mybir.dt.uint32
U8 = mybir.dt.uint8
I32 = mybir.dt.int32

V, E, H, B, S = 32000, 512, 512, 16, 128
NCORES = 8
VS = V // NCORES
HS = H // NCORES
NT = 8
NTW = VS // NT

_cache = {}


def _build(T_steps):
    nc = bacc.Bacc("TRN2", target_bir_lowering=False, debug=False, num_devices=NCORES)

    embedding = nc.dram_tensor("embedding", [V, E], F32, kind="ExternalInput")
    encT_d = nc.dram_tensor("encT", [H, 2, S], F32, kind="ExternalInput")
    enc_d = nc.dram_tensor("enc", [S, 2, H], F32, kind="ExternalInput")
    hT_d = nc.dram_tensor("hT", [2, H, B], F32, kind="ExternalInput")
    cT_d = nc.dram_tensor("cT", [2, HS, B], F32, kind="ExternalInput")
    wih0_d = nc.dram_tensor("wih0", [8, 128, 256], F32, kind="ExternalInput")
    whh0_d = nc.dram_tensor("whh0", [4, 128, 256], F32, kind="ExternalInput")
    wih1_d = nc.dram_tensor("wih1", [4, 128, 256], F32, kind="ExternalInput")
    whh1_d = nc.dram_tensor("whh1", [4, 128, 256], F32, kind="ExternalInput")
    bias0_d = nc.dram_tensor("bias0", [256, 1], F32, kind="ExternalInput")
    bias1_d = nc.dram_tensor("bias1", [256, 1], F32, kind="ExternalInput")
    outwT_d = nc.dram_tensor("outwT", [4, 128, VS], F32, kind="ExternalInput")
    outb_d = nc.dram_tensor("outb", [B, VS], F32, kind="ExternalInput")
    ident_d = nc.dram_tensor("ident", [B, B], F32, kind="ExternalInput")
    onehot_d = nc.dram_tensor("onehot", [B, 2], F32, kind="ExternalInput")
    voff_d = nc.dram_tensor("voff", [B, 1], F32, kind="ExternalInput")

    logp_o = nc.dram_tensor("logp", [T_steps, B, VS], F32, kind="ExternalOutput")
    seq_o = nc.dram_tensor("seq", [B, T_steps], I32, kind="ExternalOutput")
    attn_o = nc.dram_tensor("attn", [2, T_steps, S], F32, kind="ExternalOutput")

    with tile.TileContext(nc) as tc:
        with tc.tile_pool(name="w", bufs=1) as wp, \
             tc.tile_pool(name="st", bufs=2) as stp, \
             tc.tile_pool(name="wk", bufs=2) as wk, \
             tc.tile_pool(name="wk1", bufs=1) as wk1, \
             tc.tile_pool(name="ps1", bufs=1, space="PSUM") as ps1, \
             tc.tile_pool(name="ps2", bufs=2, space="PSUM") as ps2, \
             tc.tile_pool(name="dr", bufs=2, space="DRAM") as dr:

            encT = wp.tile([128, 4, 2, S], F32)
            nc.sync.dma_start(encT[:], encT_d.ap().rearrange("(c p) b s -> p c b s", p=128))
            encn = wp.tile([128, 2, H], F32)
            nc.sync.dma_start(encn[:], enc_d.ap())
            wih0 = wp.tile([128, 8, 256], F32)
            nc.sync.dma_start(wih0[:], wih0_d.ap().rearrange("c p m -> p c m"))
            whh0 = wp.tile([128, 4, 256], F32)
            nc.sync.dma_start(whh0[:], whh0_d.ap().rearrange("c p m -> p c m"))
            wih1 = wp.tile([128, 4, 256], F32)
            nc.sync.dma_start(wih1[:], wih1_d.ap().rearrange("c p m -> p c m"))
            whh1 = wp.tile([128, 4, 256], F32)
            nc.sync.dma_start(whh1[:], whh1_d.ap().rearrange("c p m -> p c m"))
            outw = wp.tile([128, 4, VS], F32)
            nc.sync.dma_start(outw[:], outwT_d.ap().rearrange("c p v -> p c v"))
            outb = wp.tile([B, VS], F32)
            nc.sync.dma_start(outb[:], outb_d.ap())
            bias0 = wp.tile([64, 4], F32)
            nc.sync.dma_start(bias0[:], bias0_d.ap().rearrange("(g p) o -> p (g o)", p=64))
            bias1 = wp.tile([64, 4], F32)
            nc.sync.dma_start(bias1[:], bias1_d.ap().rearrange("(g p) o -> p (g o)", p=64))
            ident = wp.tile([B, B], F32)
            nc.sync.dma_start(ident[:], ident_d.ap())
            onehot = wp.tile([B, 2], F32)
            nc.sync.dma_start(onehot[:], onehot_d.ap())
            voff = wp.tile([B, 1], F32)
            nc.sync.dma_start(voff[:], voff_d.ap())
            inf_t = wp.tile([B, 8], F32)
            nc.vector.memset(inf_t[:], 3.0e9)
            seq_sb = wp.tile([B, T_steps], F32)

            h0T = stp.tile([128, 4, B], F32, tag="h0T")
            nc.sync.dma_start(h0T[:], hT_d.ap()[0].rearrange("(c p) b -> p c b", p=128))
            h1T = stp.tile([128, 4, B], F32, tag="h1T")
            nc.sync.dma_start(h1T[:], hT_d.ap()[1].rearrange("(c p) b -> p c b", p=128))
            c0 = stp.tile([HS, B], F32, tag="c0")
            nc.sync.dma_start(c0[:], cT_d.ap()[0])
            c1 = stp.tile([HS, B], F32, tag="c1")
            nc.sync.dma_start(c1[:], cT_d.ap()[1])
            tok = stp.tile([B, 1], U32, tag="tok")
            nc.vector.memset(tok[:], 1)

            for t in range(T_steps):
                # embedding gather (carry token) + transpose to [h, b]
                emb = wk.tile([B, E], F32, tag="emb")
                nc.gpsimd.indirect_dma_start(
                    out=emb[:], out_offset=None, in_=embedding.ap(),
                    in_offset=bass.IndirectOffsetOnAxis(ap=tok[:, :1], axis=0))
                embT = wk.tile([128, 4, B], F32, tag="embT")
                for c in range(4):
                    tp = ps2.tile([128, B], F32, tag="g")
                    nc.tensor.transpose(out=tp[:], in_=emb[:, c * 128:(c + 1) * 128],
                                        identity=ident[:])
                    nc.vector.tensor_copy(embT[:, c, :], tp[:])

                # attention (query = h1 carry), batch-sharded: local rows 2k, 2k+1
                aw_list = []
                for j in range(2):
                    sc_ps = ps1.tile([B, S], F32, tag="a")
                    for c in range(4):
                        nc.tensor.matmul(sc_ps[:], h1T[:, c, :], encT[:, c, j, :],
                                         start=(c == 0), stop=(c == 3))
                    sc_sb = wk.tile([B, S], F32, tag="scsb")
                    nc.vector.tensor_copy(sc_sb[:], sc_ps[:])
                    row = ps1.tile([1, S], F32, tag="row")
                    nc.tensor.matmul(row[:], onehot[:, j:j + 1], sc_sb[:], start=True, stop=True)
                    mx = wk.tile([1, 1], F32, tag="amx")
                    nc.vector.tensor_reduce(mx[:], row[:], axis=mybir.AxisListType.X, op=ALU.max)
                    nmx = wk.tile([1, 1], F32, tag="anmx")
                    nc.vector.tensor_scalar_mul(nmx[:], mx[:], -1.0)
                    ex = wk.tile([1, S], F32, tag="aex")
                    sume = wk.tile([1, 1], F32, tag="asum")
                    nc.scalar.activation(ex[:], row[:], AF.Exp, bias=nmx[:, :1], scale=1.0,
                                         accum_out=sume[:, :1])
                    rs = wk.tile([1, 1], F32, tag="ars")
                    nc.vector.reciprocal(rs[:], sume[:])
                    aw = wk.tile([1, S], F32, tag=f"aw{j}")
                    nc.vector.tensor_scalar_mul(aw[:], ex[:], rs[:, :1])
                    nc.sync.dma_start(attn_o.ap()[j, t, :], aw[:])
                    aw_list.append(aw)
                ctxp = ps2.tile([128, 8], F32, tag="g")
                for j in range(2):
                    awT_ps = ps1.tile([S, 1], F32, tag="awT")
                    nc.tensor.transpose(out=awT_ps[:], in_=aw_list[j][:], identity=ident[:1, :1])
                    awT = wk.tile([S, 1], F32, tag="awTs")
                    nc.vector.tensor_copy(awT[:], awT_ps[:])
                    for c in range(4):
                        nc.tensor.matmul(ctxp[:, c * 2 + j:c * 2 + j + 1],
                                         encn[:, j, c * 128:(c + 1) * 128],
                                         awT[:], start=True, stop=True)
                ctxl = wk.tile([128, 8], F32, tag="ctxl")
                nc.vector.tensor_copy(ctxl[:], ctxp[:])
                e1i = dr.tile([128, 8], F32, tag="e1i")
                e1o = dr.tile([1024, 8], F32, tag="e1o")
                nc.sync.dma_start(e1i[:], ctxl[:])
                nc.gpsimd.collective_compute("AllGather", ALU.bypass,
                                             replica_groups=[list(range(NCORES))],
                                             ins=[e1i.opt()], outs=[e1o.opt()])
                ctxT = wk.tile([128, 8, 4, 2], F32, tag="ctxT")
                nc.sync.dma_start(ctxT[:], e1o[:].rearrange("(k p) f -> p k f", p=128))

                # LSTM layer 0 (this core's 64 h-dims; per-gate M=64 tiles)
                h0l = wk.tile([HS, B], F32, tag="h0l")
                c0n = stp.tile([HS, B], F32, tag="c0")
                act0 = []
                for g in range(4):
                    gp = ps2.tile([HS, B], F32, tag="g")
                    for c in range(4):
                        nc.tensor.matmul(gp[:], wih0[:, c, g * HS:(g + 1) * HS],
                                         embT[:, c, :], start=(c == 0), stop=False)
                    for c in range(4):
                        nc.tensor.matmul(gp[:], wih0[:, 4 + c, g * HS:(g + 1) * HS],
                                         ctxT[:, :, c, :], start=False, stop=False)
                    for c in range(4):
                        nc.tensor.matmul(gp[:], whh0[:, c, g * HS:(g + 1) * HS],
                                         h0T[:, c, :], start=False, stop=(c == 3))
                    a = wk.tile([HS, B], F32, tag=f"act0{g}")
                    fn = AF.Tanh if g == 2 else AF.Sigmoid
                    nc.scalar.activation(a[:], gp[:], fn, bias=bias0[:, g:g + 1])
                    act0.append(a)
                t1 = wk.tile([HS, B], F32, tag="t1")
                nc.vector.tensor_mul(t1[:], act0[1][:], c0[:])
                t2 = wk.tile([HS, B], F32, tag="t2")
                nc.vector.tensor_mul(t2[:], act0[0][:], act0[2][:])
                nc.vector.tensor_add(c0n[:], t1[:], t2[:])
                c0 = c0n
                tc0 = wk.tile([HS, B], F32, tag="tc0")
                nc.scalar.activation(tc0[:], c0[:], AF.Tanh)
                nc.vector.tensor_mul(h0l[:], act0[3][:], tc0[:])
                e2i = dr.tile([HS, B], F32, tag="e2i")
                e2o = dr.tile([H, B], F32, tag="e2o")
                nc.sync.dma_start(e2i[:], h0l[:])
                nc.gpsimd.collective_compute("AllGather", ALU.bypass,
                                             replica_groups=[list(range(NCORES))],
                                             ins=[e2i.opt()], outs=[e2o.opt()])
                h0T = stp.tile([128, 4, B], F32, tag="h0T")
                nc.sync.dma_start(h0T[:], e2o[:].rearrange("(c p) b -> p c b", p=128))

                # LSTM layer 1
                h1l = wk.tile([HS, B], F32, tag="h1l")
                c1n = stp.tile([HS, B], F32, tag="c1")
                act1 = []
                for g in range(4):
                    gp = ps2.tile([HS, B], F32, tag="g")
                    for c in range(4):
                        nc.tensor.matmul(gp[:], whh1[:, c, g * HS:(g + 1) * HS],
                                         h1T[:, c, :], start=(c == 0), stop=False)
                    for c in range(4):
                        nc.tensor.matmul(gp[:], wih1[:, c, g * HS:(g + 1) * HS],
                                         h0T[:, c, :], start=False, stop=(c == 3))
                    a = wk.tile([HS, B], F32, tag=f"act1{g}")
                    fn = AF.Tanh if g == 2 else AF.Sigmoid
                    nc.scalar.activation(a[:], gp[:], fn, bias=bias1[:, g:g + 1])
                    act1.append(a)
                t3 = wk.tile([HS, B], F32, tag="t3")
                nc.vector.tensor_mul(t3[:], act1[1][:], c1[:])
                t4 = wk.tile([HS, B], F32, tag="t4")
                nc.vector.tensor_mul(t4[:], act1[0][:], act1[2][:])
                nc.vector.tensor_add(c1n[:], t3[:], t4[:])
                c1 = c1n
                tc1 = wk.tile([HS, B], F32, tag="tc1")
                nc.scalar.activation(tc1[:], c1[:], AF.Tanh)
                nc.vector.tensor_mul(h1l[:], act1[3][:], tc1[:])
                e3i = dr.tile([HS, B], F32, tag="e3i")
                e3o = dr.tile([H, B], F32, tag="e3o")
                nc.sync.dma_start(e3i[:], h1l[:])
                nc.gpsimd.collective_compute("AllGather", ALU.bypass,
                                             replica_groups=[list(range(NCORES))],
                                             ins=[e3i.opt()], outs=[e3o.opt()])
                h1T = stp.tile([128, 4, B], F32, tag="h1T")
                nc.sync.dma_start(h1T[:], e3o[:].rearrange("(c p) b -> p c b", p=128))

                # vocab projection (shard) from NEW h1, local then global argmax
                logits = wk1.tile([B, VS], F32, tag="logits")
                for n in range(NT):
                    vp = ps2.tile([B, NTW], F32, tag="v")
                    for c in range(4):
                        nc.tensor.matmul(vp[:], h1T[:, c, :],
                                         outw[:, c, n * NTW:(n + 1) * NTW],
                                         start=(c == 0), stop=(c == 3))
                    nc.vector.tensor_add(logits[:, n * NTW:(n + 1) * NTW], vp[:],
                                         outb[:, n * NTW:(n + 1) * NTW])
                vmax = wk.tile([B, 8], F32, tag="vmax")
                vidx = wk.tile([B, 8], U32, tag="vidx")
                nc.vector.max_with_indices(vmax[:], vidx[:], logits[:])
                m_l = vmax[:, :1]
                nml = wk.tile([B, 1], F32, tag="nml")
                nc.vector.tensor_scalar_mul(nml[:], m_l, -1.0)
                logp_sb = wk1.tile([B, VS], F32, tag="logp_sb")
                sumel = wk.tile([B, 1], F32, tag="sumel")
                nc.scalar.activation(logp_sb[:], logits[:], AF.Exp, bias=nml[:, :1],
                                     scale=1.0, accum_out=sumel[:, :1])
                gidx = wk.tile([B, 1], F32, tag="gidx")
                vidxf = wk.tile([B, 1], F32, tag="vidxf")
                nc.vector.tensor_copy(vidxf[:], vidx[:, :1])
                nc.vector.tensor_add(gidx[:], vidxf[:], voff[:])
                pay = wk.tile([B, 4], F32, tag="pay")
                nc.vector.tensor_copy(pay[:, 0:1], m_l)
                nc.vector.tensor_copy(pay[:, 1:2], gidx[:])
                nc.vector.tensor_copy(pay[:, 2:3], sumel[:])
                e4i = dr.tile([B, 4], F32, tag="e4i")
                e4o = dr.tile([B * NCORES, 4], F32, tag="e4o")
                nc.sync.dma_start(e4i[:], pay[:])
                nc.gpsimd.collective_compute("AllGather", ALU.bypass,
                                             replica_groups=[list(range(NCORES))],
                                             ins=[e4i.opt()], outs=[e4o.opt()])
                mi = wk.tile([B, NCORES, 4], F32, tag="mi")
                nc.sync.dma_start(mi[:], e4o[:].rearrange("(k b) f -> b k f", b=B))

                m_g = wk.tile([B, 1], F32, tag="m_g")
                nc.vector.tensor_reduce(m_g[:], mi[:, :, 0], axis=mybir.AxisListType.X, op=ALU.max)
                mask = wk.tile([B, NCORES], U8, tag="mask")
                nc.vector.tensor_scalar(mask[:], mi[:, :, 0], m_g[:, :1], None, op0=ALU.is_equal)
                self_idx = wk.tile([B, NCORES], F32, tag="selidx")
                nc.vector.select(self_idx[:], mask[:], mi[:, :, 1], inf_t[:])
                tokf = wk.tile([B, 1], F32, tag="tokf")
                nc.vector.tensor_reduce(tokf[:], self_idx[:], axis=mybir.AxisListType.X, op=ALU.min)
                nc.vector.tensor_copy(seq_sb[:, t:t + 1], tokf[:])
                tok = stp.tile([B, 1], U32, tag="tok")
                nc.vector.tensor_copy(tok[:], tokf[:])
                dmk = wk.tile([B, NCORES], F32, tag="dmk")
                nc.vector.tensor_scalar_sub(dmk[:], mi[:, :, 0], m_g[:, :1])
                edk = wk.tile([B, NCORES], F32, tag="edk")
                nc.scalar.activation(edk[:], dmk[:], AF.Exp)
                wke = wk.tile([B, NCORES], F32, tag="wke")
                nc.vector.tensor_mul(wke[:], edk[:], mi[:, :, 2])
                ssum = wk.tile([B, 1], F32, tag="ssum")
                nc.vector.tensor_reduce(ssum[:], wke[:], axis=mybir.AxisListType.X, op=ALU.add)
                lns = wk.tile([B, 1], F32, tag="lns")
                nc.scalar.activation(lns[:], ssum[:], AF.Ln)
                nlz = wk.tile([B, 1], F32, tag="nlz")
                nc.vector.tensor_add(nlz[:], lns[:], m_g[:])
                nc.vector.tensor_scalar_mul(nlz[:], nlz[:], -1.0)
                nc.scalar.activation(logp_sb[:], logits[:], AF.Identity, bias=nlz[:, :1], scale=1.0)
                nc.sync.dma_start(logp_o.ap()[t], logp_sb[:])

            seq_i = wk.tile([B, T_steps], I32, tag="seq_i")
            nc.vector.tensor_copy(seq_i[:], seq_sb[:])
            nc.sync.dma_start(seq_o.ap(), seq_i[:])

    nc.compile()
    return nc


def _make_core_inputs(I, k):
    emb = np.ascontiguousarray(np.asarray(I["embedding"], np.float32))
    enc = np.ascontiguousarray(np.asarray(I["encoder_outputs"], np.float32))
    encT = np.ascontiguousarray(enc.transpose(2, 1, 0))
    bloc = [2 * k, 2 * k + 1]
    rows = np.concatenate([np.arange(64 * k, 64 * k + 64) + 512 * g for g in range(4)])

    def packw(w):
        wt = np.asarray(w, np.float32).T[:, rows]
        return np.ascontiguousarray(wt.reshape(-1, 128, 256))

    return {
        "embedding": emb,
        "encT": np.ascontiguousarray(encT[:, bloc, :]),
        "enc": np.ascontiguousarray(enc[:, bloc, :]),
        "hT": np.ascontiguousarray(np.asarray(I["h"], np.float32).transpose(0, 2, 1)),
        "cT": np.ascontiguousarray(
            np.asarray(I["c"], np.float32).transpose(0, 2, 1)[:, 64 * k:64 * k + 64, :]),
        "wih0": packw(I["w_ih0"]), "whh0": packw(I["w_hh0"]),
        "wih1": packw(I["w_ih1"]), "whh1": packw(I["w_hh1"]),
        "bias0": (np.asarray(I["b_ih0"], np.float32)
                  + np.asarray(I["b_hh0"], np.float32))[rows].reshape(256, 1),
        "bias1": (np.asarray(I["b_ih1"], np.float32)
                  + np.asarray(I["b_hh1"], np.float32))[rows].reshape(256, 1),
        "outwT": np.ascontiguousarray(
            np.asarray(I["out_w"], np.float32).T[:, VS * k:VS * (k + 1)].reshape(4, 128, VS)),
        "outb": np.ascontiguousarray(
            np.broadcast_to(np.asarray(I["out_b"], np.float32)[VS * k:VS * (k + 1)], (B, VS))),
        "ident": np.eye(B, dtype=np.float32),
        "onehot": np.ascontiguousarray(np.eye(B, dtype=np.float32)[:, bloc]),
        "voff": np.full((B, 1), VS * k, np.float32),
    }


def kernel(**inputs):
    T_steps = int(np.asarray(inputs["encoder_outputs"]).shape[0])
    key = T_steps
    if key not in _cache:
        _cache[key] = _build(T_steps)
    nc = _cache[key]
    in_maps = [_make_core_inputs(inputs, k) for k in range(NCORES)]
    res = run_bass_kernel_spmd(nc, in_maps, core_ids=list(range(NCORES)))
    results = res.results
    logp = np.concatenate([r["logp"] for r in results], axis=2)
    logp = np.ascontiguousarray(logp.transpose(1, 0, 2))
    seq = np.ascontiguousarray(results[0]["seq"])
    attn = np.concatenate([r["attn"] for r in results], axis=0)
    return (logp.astype(np.float32), seq.astype(np.int32), attn.astype(np.float32))
